# revision 10
# baseline (speedup 1.0000x reference)
"""Trainium2 Bass kernel for KANPolyLayer:
    y[b,o] = sum_{i,p} x[b,i]^p * coeffs[o,i,p] + bias[o],  p = 0..4

Math: y = sum_{p=1..4} (x^p) @ C_p^T + (bias + colsum(C_0)), with
C_p = coeffs[:, :, p].  Implemented as 4 accumulated GEMM planes in
float32r (FP22 truncated fp32, full PE rate) with powers computed
on-chip (ScalarE square + VectorE muls).

Per-core schedule: the x^p power slabs ([i, b] layout) are resident in
SBUF; coefficient tiles stream through a small ring.  All 8 output
groups (4 o-tiles x 2 b-halves) accumulate concurrently in 8 PSUM
banks, so each arriving coefficient tile immediately unlocks 8 matmuls
and the PE never waits on the 10 MB coefficient stream.  The p=0
constant column and bias are reduced on-device with small matmuls into
a PSUM column, then applied as a per-partition scalar during the
PSUM->SBUF copy.  The kernel computes yT = [o, b]; host transposes.

Sharding (8 cores): 4 batch groups x 2 out-dim groups.
  core c -> (bg, og) = (c // 2, c % 2)
  per-core x slice:    rows [bg*1024, (bg+1)*1024)   (transposed on host)
  per-core out slice:  cols [og*512, (og+1)*512)
Each core computes a disjoint (512 x 1024) block of yT; host gathers.
"""

from contextlib import ExitStack

import numpy as np

import concourse.bacc as bacc
import concourse.bass as bass
import concourse.mybir as mybir
import concourse.tile as tile
from concourse.bass_utils import run_bass_kernel_spmd

F32 = mybir.dt.float32
F32R = mybir.dt.float32r

B, I, O = 4096, 1024, 1024  # batch, in_dim, out_dim
BW, OW = 4, 2               # batch groups x out-dim groups (8 cores)
BS, OS = B // BW, O // OW   # per-core batch (1024) and out (512)
NK = I // 128               # contraction tiles (8)
NT = OS // 128              # o-tiles (4)
NH = BS // 512              # b-halves (2)

_CACHE: dict = {}


def _build():
    nc = bacc.Bacc("TRN2", target_bir_lowering=False, debug=False, num_devices=8)

    xt = nc.dram_tensor("xt", [I, BS], F32, kind="ExternalInput")      # [i, b]
    ct = nc.dram_tensor("ct", [4, I, OS], F32, kind="ExternalInput")   # [p-1, i, o]
    c0o = nc.dram_tensor("c0o", [OS, I], F32, kind="ExternalInput")    # [o, i]
    biasc = nc.dram_tensor("biasc", [OS, 1], F32, kind="ExternalInput")
    yt = nc.dram_tensor("yt", [OS, BS], F32, kind="ExternalOutput")    # [o, b]

    NTAIL = 2  # trailing k-planes emitted group-contiguous (tail stagger)

    with tile.TileContext(nc) as tc, ExitStack() as ctx:
        cons = ctx.enter_context(tc.tile_pool(name="cons", bufs=1))
        c0pool = ctx.enter_context(tc.tile_pool(name="c0", bufs=4))
        cpool = ctx.enter_context(tc.tile_pool(name="coef", bufs=12))
        ppool = ctx.enter_context(tc.tile_pool(name="pow", bufs=1))
        opool = ctx.enter_context(tc.tile_pool(name="out", bufs=3))
        pspool = ctx.enter_context(
            tc.tile_pool(name="ps", bufs=8, space=bass.MemorySpace.PSUM)
        )

        # 8 concurrent accumulation groups: (o-tile, b-half) -> one PSUM bank
        ps = {}
        for ot in range(NT):
            for h in range(NH):
                ps[(ot, h)] = pspool.tile(
                    [128, 512], F32, tag="ps", name=f"ps_{ot}_{h}"
                )

        # PE warmup: garbage matmuls on a memset tile while the first input
        # DMAs are in flight, so the HAM clock-gate reaches 2.4 GHz before
        # the real stream starts (saves the ~2us cold-start penalty).
        wz = cons.tile([128, 512], F32)
        nc.vector.memset(wz[:], 0.0)
        wr = cons.tile([128, 512], F32R)
        nc.vector.tensor_copy(wr[:], wz[:])
        for w in range(24):
            nc.tensor.matmul(
                ps[(0, 0)][:, 0:256], wr[:, 0:128], wr[:, 0:256], start=True, stop=True,
                skip_group_check=True,
            )

        pows = {}
        cpts = {}
        for k in range(NK):
            tail_k = k >= NK - NTAIL
            # k0: coefficient tile first (smaller -> lands first)
            if k == 0:
                cpt = cpool.tile([128, OS], F32R, tag="cp", name="cpt_0_1")
                nc.sync.dma_start(cpt[:], ct[0, 0:128, :].bitcast(F32R))
                cpts[(0, 1)] = cpt
            # resident power slabs [i=128, b=BS] for this k
            x1 = ppool.tile([128, BS], F32R, tag=f"p1_{k}", name=f"x1_{k}")
            nc.sync.dma_start(x1[:], xt[k * 128:(k + 1) * 128, :].bitcast(F32R))
            p2 = ppool.tile([128, BS], F32R, tag=f"p2_{k}", name=f"p2_{k}")
            p3 = ppool.tile([128, BS], F32R, tag=f"p3_{k}", name=f"p3_{k}")
            p4 = ppool.tile([128, BS], F32R, tag=f"p4_{k}", name=f"p4_{k}")
            nc.scalar.square(p2[:], x1[:])
            nc.vector.tensor_mul(p3[:], p2[:], x1[:])
            nc.vector.tensor_mul(p4[:], p2[:], p2[:])
            pows[k] = {1: x1, 2: p2, 3: p3, 4: p4}

            for p in range(1, 5):
                if (k, p) not in cpts:
                    cpt = cpool.tile(
                        [128, OS], F32R, tag="cp", name=f"cpt_{k}_{p}"
                    )
                    nc.sync.dma_start(
                        cpt[:], ct[p - 1, k * 128:(k + 1) * 128, :].bitcast(F32R)
                    )
                    cpts[(k, p)] = cpt
                if not tail_k:
                    for ot in range(NT):
                        for h in range(NH):
                            nc.tensor.matmul(
                                ps[(ot, h)],
                                cpts[(k, p)][:, ot * 128:(ot + 1) * 128],
                                pows[k][p][:, h * 512:(h + 1) * 512],
                                start=(k == 0 and p == 1),
                                stop=False,
                            )

        # bias/C0 inputs stream behind the main inputs (only needed at end):
        # biascol[o-part, ot] = bias[o] + sum_i C0[i, o], DVE-only.
        red = cons.tile([128, NT], F32)
        for ot in range(NT):
            c0s = c0pool.tile([128, I], F32, tag="c0", name=f"c0s_{ot}")
            nc.sync.dma_start(c0s[:], c0o[ot * 128:(ot + 1) * 128, :])
            nc.vector.tensor_reduce(
                red[:, ot:ot + 1], c0s[:], mybir.AxisListType.X, mybir.AluOpType.add
            )
        biasc_sb = cons.tile([128, NT], F32)
        for ot in range(NT):
            nc.sync.dma_start(
                biasc_sb[:, ot:ot + 1], biasc[ot * 128:(ot + 1) * 128, :]
            )
        biascol = cons.tile([128, NT], F32)
        nc.vector.tensor_add(biascol[:], red[:], biasc_sb[:])

        # trailing k-planes group-contiguous: each group finishes ~2.1us
        # apart, so bias-add + output DMA overlap the matmul stream
        for ot in range(NT):
            for h in range(NH):
                for k in range(NK - NTAIL, NK):
                    for p in range(1, 5):
                        nc.tensor.matmul(
                            ps[(ot, h)],
                            cpts[(k, p)][:, ot * 128:(ot + 1) * 128],
                            pows[k][p][:, h * 512:(h + 1) * 512],
                            start=False,
                            stop=(k == NK - 1 and p == 4),
                        )
                # bias-add split across both engines, halves DMA'd separately
                o_sb = opool.tile([128, 512], F32, tag="o_sb", name=f"o_{ot}_{h}")
                nc.scalar.activation(
                    o_sb[:, 0:256],
                    ps[(ot, h)][:, 0:256],
                    mybir.ActivationFunctionType.Identity,
                    bias=biascol[:, ot:ot + 1],
                )
                nc.vector.tensor_scalar_add(
                    o_sb[:, 256:512], ps[(ot, h)][:, 256:512], biascol[:, ot:ot + 1]
                )
                nc.sync.dma_start(
                    yt[ot * 128:(ot + 1) * 128, h * 512:h * 512 + 256],
                    o_sb[:, 0:256],
                )
                nc.sync.dma_start(
                    yt[ot * 128:(ot + 1) * 128, h * 512 + 256:(h + 1) * 512],
                    o_sb[:, 256:512],
                )

    nc.compile()
    return nc


def _get_nc():
    if "nc" not in _CACHE:
        _CACHE["nc"] = _build()
    return _CACHE["nc"]


def _make_in_maps(x, coeffs, bias):
    x = np.asarray(x, dtype=np.float32)
    coeffs = np.asarray(coeffs, dtype=np.float32)
    bias = np.asarray(bias, dtype=np.float32)

    xts = [
        np.ascontiguousarray(x[bg * BS:(bg + 1) * BS, :].T) for bg in range(BW)
    ]
    cts = [
        np.ascontiguousarray(
            coeffs[og * OS:(og + 1) * OS, :, 1:].transpose(2, 1, 0)
        )
        for og in range(OW)
    ]
    c0os = [
        np.ascontiguousarray(coeffs[og * OS:(og + 1) * OS, :, 0])
        for og in range(OW)
    ]
    in_maps = []
    for c in range(BW * OW):
        bg, og = c // OW, c % OW
        in_maps.append(
            {
                "xt": xts[bg],
                "ct": cts[og],
                "c0o": c0os[og],
                "biasc": np.ascontiguousarray(
                    bias[0, og * OS:(og + 1) * OS].reshape(OS, 1)
                ),
            }
        )
    return in_maps


def _gather(results):
    y = np.empty((B, O), dtype=np.float32)
    for c, res in enumerate(results):
        bg, og = c // OW, c % OW
        y[bg * BS:(bg + 1) * BS, og * OS:(og + 1) * OS] = res["yt"].T
    return y


def run(x, coeffs, bias, trace=False, **trace_kwargs):
    nc = _get_nc()
    in_maps = _make_in_maps(x, coeffs, bias)
    br = run_bass_kernel_spmd(
        nc, in_maps, list(range(BW * OW)), trace=trace, **trace_kwargs
    )
    return _gather(br.results), br


def kernel(x, coeffs, bias):
    out, _ = run(x, coeffs, bias)
    return out


# revision 11
# speedup vs baseline: 1.0064x; 1.0064x over previous
"""Trainium2 Bass kernel for KANPolyLayer:
    y[b,o] = sum_{i,p} x[b,i]^p * coeffs[o,i,p] + bias[o],  p = 0..4

Math: y = sum_{p=1..4} (x^p) @ C_p^T + (bias + colsum(C_0)), with
C_p = coeffs[:, :, p].  Implemented as 4 accumulated GEMM planes in
float32r (FP22 truncated fp32, full PE rate) with powers computed
on-chip (ScalarE square + VectorE muls).

Per-core schedule: the x^p power slabs ([i, b] layout) are resident in
SBUF; coefficient tiles stream through a small ring.  All 8 output
groups (4 o-tiles x 2 b-halves) accumulate concurrently in 8 PSUM
banks, so each arriving coefficient tile immediately unlocks 8 matmuls
and the PE never waits on the 10 MB coefficient stream.  The p=0
constant column and bias are reduced on-device with small matmuls into
a PSUM column, then applied as a per-partition scalar during the
PSUM->SBUF copy.  The kernel computes yT = [o, b]; host transposes.

Sharding (8 cores): 4 batch groups x 2 out-dim groups.
  core c -> (bg, og) = (c // 2, c % 2)
  per-core x slice:    rows [bg*1024, (bg+1)*1024)   (transposed on host)
  per-core out slice:  cols [og*512, (og+1)*512)
Each core computes a disjoint (512 x 1024) block of yT; host gathers.
"""

from contextlib import ExitStack

import numpy as np

import concourse.bacc as bacc
import concourse.bass as bass
import concourse.mybir as mybir
import concourse.tile as tile
from concourse.bass_utils import run_bass_kernel_spmd

F32 = mybir.dt.float32
F32R = mybir.dt.float32r

B, I, O = 4096, 1024, 1024  # batch, in_dim, out_dim
BW, OW = 4, 2               # batch groups x out-dim groups (8 cores)
BS, OS = B // BW, O // OW   # per-core batch (1024) and out (512)
NK = I // 128               # contraction tiles (8)
NT = OS // 128              # o-tiles (4)
NH = BS // 512              # b-halves (2)

_CACHE: dict = {}


def _build():
    nc = bacc.Bacc("TRN2", target_bir_lowering=False, debug=False, num_devices=8)

    xt = nc.dram_tensor("xt", [I, BS], F32, kind="ExternalInput")      # [i, b]
    ct = nc.dram_tensor("ct", [4, I, OS], F32, kind="ExternalInput")   # [p-1, i, o]
    c0o = nc.dram_tensor("c0o", [OS, I], F32, kind="ExternalInput")    # [o, i]
    biasc = nc.dram_tensor("biasc", [OS, 1], F32, kind="ExternalInput")
    yt = nc.dram_tensor("yt", [OS, BS], F32, kind="ExternalOutput")    # [o, b]

    NTAIL = 2  # trailing k-planes emitted group-contiguous (tail stagger)

    with tile.TileContext(nc) as tc, ExitStack() as ctx:
        cons = ctx.enter_context(tc.tile_pool(name="cons", bufs=1))
        c0pool = ctx.enter_context(tc.tile_pool(name="c0", bufs=4))
        cpool = ctx.enter_context(tc.tile_pool(name="coef", bufs=12))
        ppool = ctx.enter_context(tc.tile_pool(name="pow", bufs=1))
        opool = ctx.enter_context(tc.tile_pool(name="out", bufs=3))
        pspool = ctx.enter_context(
            tc.tile_pool(name="ps", bufs=8, space=bass.MemorySpace.PSUM)
        )

        # 8 concurrent accumulation groups: (o-tile, b-half) -> one PSUM bank
        ps = {}
        for ot in range(NT):
            for h in range(NH):
                ps[(ot, h)] = pspool.tile(
                    [128, 512], F32, tag="ps", name=f"ps_{ot}_{h}"
                )

        # PE warmup: garbage matmuls on a memset tile while the first input
        # DMAs are in flight, so the HAM clock-gate reaches 2.4 GHz before
        # the real stream starts (saves the ~2us cold-start penalty).
        wz = cons.tile([128, 512], F32)
        nc.vector.memset(wz[:], 0.0)
        wr = cons.tile([128, 512], F32R)
        nc.vector.tensor_copy(wr[:], wz[:])
        for w in range(18):
            nc.tensor.matmul(
                ps[(0, 0)][:, 0:256], wr[:, 0:128], wr[:, 0:256], start=True, stop=True,
                skip_group_check=True,
            )

        pows = {}
        cpts = {}
        for k in range(NK):
            tail_k = k >= NK - NTAIL
            # k0: coefficient tile first (smaller -> lands first)
            if k == 0:
                cpt = cpool.tile([128, OS], F32R, tag="cp", name="cpt_0_1")
                nc.sync.dma_start(cpt[:], ct[0, 0:128, :].bitcast(F32R))
                cpts[(0, 1)] = cpt
            # resident power tiles [i=128, b=512] per b-half for this k;
            # separate tiles per half so the first matmuls only wait on
            # half the x DMA bytes
            pk = {}
            for h2 in range(NH):
                x1 = ppool.tile([128, 512], F32R, tag=f"p1_{k}_{h2}",
                                name=f"x1_{k}_{h2}")
                nc.sync.dma_start(
                    x1[:],
                    xt[k * 128:(k + 1) * 128,
                       h2 * 512:(h2 + 1) * 512].bitcast(F32R),
                )
                p2 = ppool.tile([128, 512], F32R, tag=f"p2_{k}_{h2}",
                                name=f"p2_{k}_{h2}")
                p3 = ppool.tile([128, 512], F32R, tag=f"p3_{k}_{h2}",
                                name=f"p3_{k}_{h2}")
                p4 = ppool.tile([128, 512], F32R, tag=f"p4_{k}_{h2}",
                                name=f"p4_{k}_{h2}")
                nc.scalar.square(p2[:], x1[:])
                nc.vector.tensor_mul(p3[:], p2[:], x1[:])
                nc.vector.tensor_mul(p4[:], p2[:], p2[:])
                pk[(1, h2)] = x1
                pk[(2, h2)] = p2
                pk[(3, h2)] = p3
                pk[(4, h2)] = p4
            pows[k] = pk

            for p in range(1, 5):
                if (k, p) not in cpts:
                    cpt = cpool.tile(
                        [128, OS], F32R, tag="cp", name=f"cpt_{k}_{p}"
                    )
                    nc.sync.dma_start(
                        cpt[:], ct[p - 1, k * 128:(k + 1) * 128, :].bitcast(F32R)
                    )
                    cpts[(k, p)] = cpt
                if not tail_k:
                    for ot in range(NT):
                        for h in range(NH):
                            nc.tensor.matmul(
                                ps[(ot, h)],
                                cpts[(k, p)][:, ot * 128:(ot + 1) * 128],
                                pows[k][(p, h)][:],
                                start=(k == 0 and p == 1),
                                stop=False,
                            )

        # bias/C0 inputs stream behind the main inputs (only needed at end):
        # biascol[o-part, ot] = bias[o] + sum_i C0[i, o], DVE-only.
        red = cons.tile([128, NT], F32)
        for ot in range(NT):
            c0s = c0pool.tile([128, I], F32, tag="c0", name=f"c0s_{ot}")
            nc.sync.dma_start(c0s[:], c0o[ot * 128:(ot + 1) * 128, :])
            nc.vector.tensor_reduce(
                red[:, ot:ot + 1], c0s[:], mybir.AxisListType.X, mybir.AluOpType.add
            )
        biasc_sb = cons.tile([128, NT], F32)
        for ot in range(NT):
            nc.sync.dma_start(
                biasc_sb[:, ot:ot + 1], biasc[ot * 128:(ot + 1) * 128, :]
            )
        biascol = cons.tile([128, NT], F32)
        nc.vector.tensor_add(biascol[:], red[:], biasc_sb[:])

        # trailing k-planes group-contiguous: each group finishes ~2.1us
        # apart, so bias-add + output DMA overlap the matmul stream
        for ot in range(NT):
            for h in range(NH):
                for k in range(NK - NTAIL, NK):
                    for p in range(1, 5):
                        nc.tensor.matmul(
                            ps[(ot, h)],
                            cpts[(k, p)][:, ot * 128:(ot + 1) * 128],
                            pows[k][(p, h)][:],
                            start=False,
                            stop=(k == NK - 1 and p == 4),
                        )
                # bias-add split across both engines, halves DMA'd separately
                o_sb = opool.tile([128, 512], F32, tag="o_sb", name=f"o_{ot}_{h}")
                nc.scalar.activation(
                    o_sb[:, 0:256],
                    ps[(ot, h)][:, 0:256],
                    mybir.ActivationFunctionType.Identity,
                    bias=biascol[:, ot:ot + 1],
                )
                nc.vector.tensor_scalar_add(
                    o_sb[:, 256:512], ps[(ot, h)][:, 256:512], biascol[:, ot:ot + 1]
                )
                nc.sync.dma_start(
                    yt[ot * 128:(ot + 1) * 128, h * 512:h * 512 + 256],
                    o_sb[:, 0:256],
                )
                nc.sync.dma_start(
                    yt[ot * 128:(ot + 1) * 128, h * 512 + 256:(h + 1) * 512],
                    o_sb[:, 256:512],
                )

    nc.compile()
    return nc


def _get_nc():
    if "nc" not in _CACHE:
        _CACHE["nc"] = _build()
    return _CACHE["nc"]


def _make_in_maps(x, coeffs, bias):
    x = np.asarray(x, dtype=np.float32)
    coeffs = np.asarray(coeffs, dtype=np.float32)
    bias = np.asarray(bias, dtype=np.float32)

    xts = [
        np.ascontiguousarray(x[bg * BS:(bg + 1) * BS, :].T) for bg in range(BW)
    ]
    cts = [
        np.ascontiguousarray(
            coeffs[og * OS:(og + 1) * OS, :, 1:].transpose(2, 1, 0)
        )
        for og in range(OW)
    ]
    c0os = [
        np.ascontiguousarray(coeffs[og * OS:(og + 1) * OS, :, 0])
        for og in range(OW)
    ]
    in_maps = []
    for c in range(BW * OW):
        bg, og = c // OW, c % OW
        in_maps.append(
            {
                "xt": xts[bg],
                "ct": cts[og],
                "c0o": c0os[og],
                "biasc": np.ascontiguousarray(
                    bias[0, og * OS:(og + 1) * OS].reshape(OS, 1)
                ),
            }
        )
    return in_maps


def _gather(results):
    y = np.empty((B, O), dtype=np.float32)
    for c, res in enumerate(results):
        bg, og = c // OW, c % OW
        y[bg * BS:(bg + 1) * BS, og * OS:(og + 1) * OS] = res["yt"].T
    return y


def run(x, coeffs, bias, trace=False, **trace_kwargs):
    nc = _get_nc()
    in_maps = _make_in_maps(x, coeffs, bias)
    br = run_bass_kernel_spmd(
        nc, in_maps, list(range(BW * OW)), trace=trace, **trace_kwargs
    )
    return _gather(br.results), br


def kernel(x, coeffs, bias):
    out, _ = run(x, coeffs, bias)
    return out
